# revision 41
# baseline (speedup 1.0000x reference)
"""ASReader kernel for Trainium2 (Bass/Tile), data-parallel over batch on 8 NeuronCores.

Per core (BL=B/8 examples):
  phase 1: q_pooled = max over query tokens (gpsimd partition reduce + broadcast),
           scores = doc @ q. Columns are split between a fused DVE
           scalar_tensor_tensor (mul+accum) path and a gpsimd mul + ACT
           accumulate path so no single engine is the bottleneck.
           Vectors [T] live as [128 p, C c] tiles with t = c*128 + p.
  phase 2: masked softmax (DVE reduce + gpsimd partition reduce, exp on ACT).
           The mask is loaded in DMA-friendly [c, p] layout and transposed
           on the PE.
  phase 3: general interval spans. Membership 1[t>=v] factorizes as
           1[c>vc] + 1[c==vc]*1[p>=vp]; span sums and token scales are built
           from disjoint nonnegative contributions (full columns / partial
           start col / partial end col, same-column spans selected via
           copy_predicated) so near-zero span sums keep reference semantics
           (no catastrophic cancellation) and uncovered tokens get exact
           zeros. Applied via small PE matmuls, then renormalize.

kernel(**inputs) takes FULL inputs, shards batch across the 8 cores, returns FULL output.
"""

from contextlib import ExitStack
from dataclasses import dataclass

import numpy as np

import concourse.bass as bass
import concourse.bacc as bacc
import concourse.mybir as mybir
import concourse.tile as tile
from concourse import bass_isa
from concourse.bass_utils import run_bass_kernel_spmd

F32 = mybir.dt.float32
I32 = mybir.dt.int32
OP = mybir.AluOpType
AF = mybir.ActivationFunctionType
AX = mybir.AxisListType
P = 128  # partitions


@dataclass(frozen=True)
class Sizes:
    BL: int = 4      # examples per core
    T: int = 4096
    TQ: int = 64
    D: int = 256
    S: int = 64
    DCH: int = 8     # doc columns per DMA chunk
    DVE_COLS: int = 5    # of each DCH group, this many columns go to the DVE path

    @property
    def C(self):     # columns of the [P, C] layout
        return self.T // P


def build_kernel(ctx: ExitStack, tc: "tile.TileContext", z: Sizes, doc, query, mask, spans, sent, out, stop_after=None):
    nc = tc.nc
    BL, T, TQ, D, S, C = z.BL, z.T, z.TQ, z.D, z.S, z.C
    DCH = z.DCH if C % z.DCH == 0 else C
    NCH = C // DCH
    NDVE = min(z.DVE_COLS, DCH)

    consts = ctx.enter_context(tc.tile_pool(name="consts", bufs=1))
    qpool = ctx.enter_context(tc.tile_pool(name="qpool", bufs=2))
    docp = ctx.enter_context(tc.tile_pool(name="docp", bufs=3))
    scr = ctx.enter_context(tc.tile_pool(name="scr", bufs=3))
    work = ctx.enter_context(tc.tile_pool(name="work", bufs=2))
    psum = ctx.enter_context(tc.tile_pool(name="psum", bufs=1, space="PSUM"))

    # ---------------- constants ----------------
    iotaC_i = consts.tile([S, C], I32, tag="iotaC_i")
    nc.gpsimd.iota(iotaC_i, pattern=[[1, C]], base=0, channel_multiplier=0)
    iotaC = consts.tile([S, C], F32, tag="iotaC")
    nc.vector.tensor_copy(iotaC, iotaC_i)

    iotaPr_i = consts.tile([S, P], I32, tag="iotaPr_i")
    nc.gpsimd.iota(iotaPr_i, pattern=[[1, P]], base=0, channel_multiplier=0)
    iotaPr = consts.tile([S, P], F32, tag="iotaPr")
    nc.vector.tensor_copy(iotaPr, iotaPr_i)

    iotaPc_i = consts.tile([P, 1], I32, tag="iotaPc_i")
    nc.gpsimd.iota(iotaPc_i, pattern=[[1, 1]], base=0, channel_multiplier=1)
    iotaPc = consts.tile([P, 1], F32, tag="iotaPc")
    nc.vector.tensor_copy(iotaPc, iotaPc_i)

    ones_col = consts.tile([P, 1], F32, tag="ones_col")
    nc.vector.memset(ones_col, 1.0)
    onesSP = consts.tile([S, P], F32, tag="onesSP")
    nc.vector.memset(onesSP, 1.0)
    zerosS = consts.tile([S, 1], F32, tag="zerosS")
    nc.vector.memset(zerosS, 0.0)
    idC = consts.tile([C, C], F32, tag="idC")     # identity for PE transpose
    nc.vector.tensor_scalar(idC, iotaC[0:C, :], iotaPc[0:C], None, op0=OP.is_equal)

    # ---------------- small input staging ----------------
    # b0's query slice first (tiny - it gates the first matvec), then the first
    # doc chunk, then the remaining staging behind them.
    query_sb = consts.tile([TQ, BL, D], F32, tag="query_sb")
    nc.sync.dma_start(out=query_sb[:, 0, :], in_=query[0])
    pre_dtile = docp.tile([P, z.DCH if C % z.DCH == 0 else C, D], F32, tag="dtile")
    nc.sync.dma_start(
        out=pre_dtile,
        in_=doc[0, 0:(z.DCH if C % z.DCH == 0 else C) * P, :].rearrange("(c p) d -> p c d", p=P),
    )
    spans_sb = consts.tile([S, BL, 2], I32, tag="spans_sb")
    nc.sync.dma_start(out=spans_sb, in_=spans.rearrange("b s e -> s b e"))
    spans_row = consts.tile([1, BL * S * 2], I32, tag="spans_row")
    sp_flat = spans[:]
    nc.sync.dma_start(
        out=spans_row,
        in_=bass.AP(tensor=sp_flat.tensor, offset=sp_flat.offset, ap=[[0, 1], [1, BL * S * 2]]),
    )
    sent_sb = consts.tile([S, BL], F32, tag="sent_sb")
    nc.sync.dma_start(out=sent_sb, in_=sent.rearrange("b s -> s b"))
    # mask in DMA-friendly [c, b, p] layout; transposed per-b on PE below
    mask_cp = consts.tile([C, BL, P], F32, tag="mask_cp")
    nc.sync.dma_start(out=mask_cp, in_=mask.rearrange("b (c p) -> c b p", p=P))
    if BL > 1:
        nc.sync.dma_start(out=query_sb[:, 1:, :], in_=query[1:].rearrange("b t d -> t b d"))

    # ---------------- q_pooled (per b): max over TQ then broadcast to 128 ----------------
    qb_tiles = []
    for b in range(BL):
        qmax = qpool.tile([TQ, D], F32, tag=f"qmax{b % 2}")
        nc.gpsimd.partition_all_reduce(qmax, query_sb[:, b, :], channels=TQ, reduce_op=bass_isa.ReduceOp.max)
        qb = qpool.tile([P, D], F32, tag=f"qb{b}")
        nc.gpsimd.partition_broadcast(qb, qmax[0:1, :])
        qb_tiles.append(qb)

    # mask transposed to [P, C] per b (PE transpose -> PSUM -> ACT copy to SBUF)
    def mask_transpose(b):
        mps = psum.tile([P, C], F32, tag="mps")
        nc.tensor.transpose(mps, mask_cp[:, b, :], idC)
        msb = prep.tile([P, C], F32, tag="msb")
        nc.scalar.copy(msb, mps)
        return msb

    def doc_dma(b, g):
        dtile = docp.tile([P, DCH, D], F32, tag="dtile")
        nc.sync.dma_start(
            out=dtile,
            in_=doc[b, g * DCH * P:(g + 1) * DCH * P, :].rearrange("(c p) d -> p c d", p=P),
        )
        return dtile

    def matvec_chunk(b, g, scores, dtile=None, split_last=False):
        qb = qb_tiles[b]
        if dtile is None:
            dtile = doc_dma(b, g)
        ndve = NDVE
        for cc in range(ndve):
            sc_out = scr.tile([P, D], F32, tag="stt_out")
            nc.vector.scalar_tensor_tensor(
                out=sc_out, in0=dtile[:, cc, :], scalar=1.0, in1=qb,
                op0=OP.mult, op1=OP.mult,
                accum_out=scores[:, g * DCH + cc: g * DCH + cc + 1],
            )
        for cc in range(ndve, DCH):
            prod = scr.tile([P, D], F32, tag="prod")
            nc.gpsimd.tensor_tensor(out=prod, in0=dtile[:, cc, :], in1=qb, op=OP.mult)
            ajunk = scr.tile([P, D], F32, tag="ajunk")
            nc.scalar.activation(
                out=ajunk, in_=prod, func=AF.Copy,
                accum_out=scores[:, g * DCH + cc: g * DCH + cc + 1],
            )

    def span_prep(b):
        """Everything in phase 3 that depends only on spans/sent (not probs).
        Runs in engine-idle gaps during the doc DMA stream."""
        pr = {}
        pr["bmask"] = mask_transpose(b)
        vc_i = prep.tile([S, 2], I32, tag="vc_i")
        nc.vector.tensor_scalar(vc_i, spans_sb[:, b, :], 7, None, op0=OP.logical_shift_right)
        vp_i = prep.tile([S, 2], I32, tag="vp_i")
        nc.vector.tensor_scalar(vp_i, spans_sb[:, b, :], 127, None, op0=OP.bitwise_and)
        vc = prep.tile([S, 2], F32, tag="vc")
        nc.vector.tensor_copy(vc, vc_i)
        vp = prep.tile([S, 2], F32, tag="vp")
        nc.vector.tensor_copy(vp, vp_i)
        pr["vc"], pr["vp"] = vc, vp
        vcS, vcE = vc[:, 0:1], vc[:, 1:2]
        vpS, vpE = vp[:, 0:1], vp[:, 1:2]

        # vp of both ends as rows -> broadcast down partitions: [P, 2S] (s,e interleaved)
        vpr_i = prep.tile([1, 2 * S], I32, tag="vpr_i")
        nc.vector.tensor_scalar(vpr_i, spans_row[0:1, b * 2 * S:(b + 1) * 2 * S], 127, None, op0=OP.bitwise_and)
        vpr = prep.tile([1, 2 * S], F32, tag="vpr")
        nc.vector.tensor_copy(vpr, vpr_i)
        vpb = prep.tile([P, 2 * S], F32, tag="vpb")
        nc.gpsimd.partition_broadcast(vpb, vpr)
        vpb2 = vpb.rearrange("p (s e) -> p e s", e=2)
        vpSb, vpEb = vpb2[:, 0, :], vpb2[:, 1, :]

        # column-space predicates [S, C]
        ohC_S = prep.tile([S, C], F32, tag="ohC_S")
        nc.vector.tensor_scalar(ohC_S, iotaC, vcS, None, op0=OP.is_equal)
        ohC_E = prep.tile([S, C], F32, tag="ohC_E")
        nc.vector.tensor_scalar(ohC_E, iotaC, vcE, None, op0=OP.is_equal)
        gtS = prep.tile([S, C], F32, tag="gtS")
        nc.vector.tensor_scalar(gtS, iotaC, vcS, None, op0=OP.is_gt)
        ltE = prep.tile([S, C], F32, tag="ltE")
        nc.vector.tensor_scalar(ltE, iotaC, vcE, None, op0=OP.is_lt)
        full = prep.tile([S, C], F32, tag="full")
        nc.vector.tensor_tensor(out=full, in0=gtS, in1=ltE, op=OP.mult)
        same = prep.tile([S, 1], F32, tag="same")
        nc.vector.tensor_tensor(out=same, in0=vcS, in1=vcE, op=OP.is_equal)
        same_i = prep.tile([S, 1], I32, tag="same_i")
        nc.vector.tensor_tensor(out=same_i, in0=vcS, in1=vcE, op=OP.is_equal)
        ohSE = prep.tile([S, C], F32, tag="ohSE")
        nc.vector.tensor_tensor(out=ohSE, in0=ohC_S, in1=ohC_E, op=OP.mult)
        notsame = prep.tile([S, 1], F32, tag="notsame")
        nc.vector.tensor_scalar(notsame, same, -1.0, 1.0, op0=OP.mult, op1=OP.add)
        pr.update(ohC_S=ohC_S, ohC_E=ohC_E, full=full, same=same, same_i=same_i,
                  ohSE=ohSE, notsame=notsame)

        # partition-space predicates [P, S] and [S, P]
        spge_S = prep.tile([P, S], F32, tag="spge_S")
        nc.vector.tensor_scalar(spge_S, vpSb, iotaPc, None, op0=OP.is_le)   # vpS <= p
        splt_E = prep.tile([P, S], F32, tag="splt_E")
        nc.vector.tensor_scalar(splt_E, vpEb, iotaPc, None, op0=OP.is_gt)   # vpE > p
        spband = prep.tile([P, S], F32, tag="spband")
        nc.vector.tensor_tensor(out=spband, in0=spge_S, in1=splt_E, op=OP.mult)
        spgeS_sp = prep.tile([S, P], F32, tag="spgeS_sp")
        nc.vector.tensor_scalar(spgeS_sp, iotaPr, vpS, None, op0=OP.is_ge)
        spltE_sp = prep.tile([S, P], F32, tag="spltE_sp")
        nc.vector.tensor_scalar(spltE_sp, iotaPr, vpE, None, op0=OP.is_lt)
        band_sp = prep.tile([S, P], F32, tag="band_sp")
        nc.vector.tensor_tensor(out=band_sp, in0=spgeS_sp, in1=spltE_sp, op=OP.mult)
        pr.update(spge_S=spge_S, splt_E=splt_E, spband=spband,
                  spgeS_sp=spgeS_sp, spltE_sp=spltE_sp, band_sp=band_sp)

        # count and 1/count
        cnt_i = prep.tile([S, 1], I32, tag="cnt_i")
        nc.vector.tensor_tensor(out=cnt_i, in0=spans_sb[:, b, 1:2], in1=spans_sb[:, b, 0:1], op=OP.subtract)
        cnt = prep.tile([S, 1], F32, tag="cnt")
        nc.vector.tensor_copy(cnt, cnt_i)
        nc.vector.tensor_scalar_max(cnt, cnt, 1.0)
        rcnt = prep.tile([S, 1], F32, tag="rcnt")
        nc.vector.reciprocal(rcnt, cnt)
        pr["rcnt"] = rcnt
        return pr

    def phases23(b, scores, pr):
        # ---------------- phase 2: masked softmax ----------------
        bmask = pr["bmask"]
        masked = work.tile([P, C], F32, tag="masked")
        nc.vector.tensor_tensor(out=masked, in0=scores, in1=bmask, op=OP.mult)
        rowmax = work.tile([P, 1], F32, tag="rowmax")
        nc.vector.tensor_reduce(out=rowmax, in_=masked, axis=AX.X, op=OP.max)
        allmax = work.tile([P, 1], F32, tag="allmax")
        nc.gpsimd.partition_all_reduce(allmax, rowmax, channels=P, reduce_op=bass_isa.ReduceOp.max)
        negM = work.tile([P, 1], F32, tag="negM")
        nc.vector.tensor_scalar_mul(negM, allmax, -1.0)
        e = work.tile([P, C], F32, tag="e")
        nc.scalar.activation(out=e, in_=masked, func=AF.Exp, bias=negM, scale=1.0)
        em = work.tile([P, C], F32, tag="em")
        denp = work.tile([P, 1], F32, tag="denp")
        nc.vector.scalar_tensor_tensor(
            out=em, in0=e, scalar=1.0, in1=bmask,
            op0=OP.mult, op1=OP.mult, accum_out=denp,
        )
        den = work.tile([P, 1], F32, tag="den")
        nc.gpsimd.partition_all_reduce(den, denp, channels=P, reduce_op=bass_isa.ReduceOp.add)
        dinv = work.tile([P, 1], F32, tag="dinv")
        nc.vector.reciprocal(dinv, den)
        probs = work.tile([P, C], F32, tag="probs")
        nc.scalar.activation(out=probs, in_=em, func=AF.Copy, scale=dinv)

        if stop_after == "softmax":
            nc.sync.dma_start(out=out[b], in_=probs)
            return

        # ---------------- phase 3: apply spans ----------------
        # partial-column sums via PE: U[s, c] = sum_p pred[p, s] * probs[p, c]
        U1 = psum.tile([S, C], F32, tag="U1")
        nc.tensor.matmul(U1, lhsT=pr["spge_S"], rhs=probs, start=True, stop=True)
        U2 = psum.tile([S, C], F32, tag="U2")
        nc.tensor.matmul(U2, lhsT=pr["splt_E"], rhs=probs, start=True, stop=True)
        U3 = psum.tile([S, C], F32, tag="U3")
        nc.tensor.matmul(U3, lhsT=pr["spband"], rhs=probs, start=True, stop=True)
        csp = psum.tile([1, C], F32, tag="csp")
        nc.tensor.matmul(csp, lhsT=ones_col, rhs=probs, start=True, stop=True)
        cs_row = work.tile([1, C], F32, tag="cs_row")
        nc.scalar.copy(cs_row, csp)
        cs_b = work.tile([S, C], F32, tag="cs_b")
        nc.gpsimd.partition_broadcast(cs_b, cs_row)

        junk = work.tile([S, C], F32, tag="junk")
        tA = work.tile([S, 1], F32, tag="tA")
        nc.vector.scalar_tensor_tensor(out=junk, in0=cs_b, scalar=1.0, in1=pr["full"],
                                       op0=OP.mult, op1=OP.mult, accum_out=tA)
        tB1 = work.tile([S, 1], F32, tag="tB1")
        nc.vector.scalar_tensor_tensor(out=junk, in0=U1, scalar=1.0, in1=pr["ohC_S"],
                                       op0=OP.mult, op1=OP.mult, accum_out=tB1)
        tB2 = work.tile([S, 1], F32, tag="tB2")
        nc.vector.scalar_tensor_tensor(out=junk, in0=U2, scalar=1.0, in1=pr["ohC_E"],
                                       op0=OP.mult, op1=OP.mult, accum_out=tB2)
        tB3 = work.tile([S, 1], F32, tag="tB3")
        nc.vector.scalar_tensor_tensor(out=junk, in0=U3, scalar=1.0, in1=pr["ohSE"],
                                       op0=OP.mult, op1=OP.mult, accum_out=tB3)

        tBn = work.tile([S, 1], F32, tag="tBn")
        nc.vector.tensor_tensor(out=tBn, in0=tB1, in1=tB2, op=OP.add)
        nc.vector.copy_predicated(tBn, pr["same_i"], tB3)   # same-column spans use the band sum
        span_sum = work.tile([S, 1], F32, tag="span_sum")
        nc.vector.tensor_tensor(out=span_sum, in0=tA, in1=tBn, op=OP.add)

        # scale = where(mean != 0, sent/mean, 1);  x = scale - 1
        mean = work.tile([S, 1], F32, tag="mean")
        nc.vector.tensor_tensor(out=mean, in0=span_sum, in1=pr["rcnt"], op=OP.mult)
        rmean = work.tile([S, 1], F32, tag="rmean")
        nc.vector.reciprocal(rmean, mean)
        x = work.tile([S, 1], F32, tag="x")
        nc.vector.tensor_tensor(out=x, in0=sent_sb[:, b:b + 1], in1=rmean, op=OP.mult)
        nc.vector.tensor_scalar_add(x, x, -1.0)
        iszero = work.tile([S, 1], I32, tag="iszero")
        nc.vector.tensor_scalar(iszero, mean, 0.0, None, op0=OP.is_equal)
        nc.vector.copy_predicated(x, iszero, zerosS)

        # token_scale-1 = G in PSUM [P, C], built from four disjoint nonnegative
        # membership pieces (full cols / start col / end col / same-col band) so
        # uncovered tokens get exact zeros (no cross-matmul cancellation).
        xn = work.tile([S, 1], F32, tag="xn")
        nc.vector.tensor_tensor(out=xn, in0=x, in1=pr["notsame"], op=OP.mult)
        xsame = work.tile([S, 1], F32, tag="xsame")
        nc.vector.tensor_tensor(out=xsame, in0=x, in1=pr["same"], op=OP.mult)

        xones = work.tile([S, P], F32, tag="xones")
        nc.vector.tensor_scalar_mul(xones, onesSP, x)
        spge_xn = work.tile([S, P], F32, tag="spge_xn")
        nc.vector.tensor_scalar_mul(spge_xn, pr["spgeS_sp"], xn)
        splt_xn = work.tile([S, P], F32, tag="splt_xn")
        nc.vector.tensor_scalar_mul(splt_xn, pr["spltE_sp"], xn)
        band_xs = work.tile([S, P], F32, tag="band_xs")
        nc.vector.tensor_scalar_mul(band_xs, pr["band_sp"], xsame)

        G = psum.tile([P, C], F32, tag="G")
        nc.tensor.matmul(G, lhsT=xones, rhs=pr["full"], start=True, stop=False)
        nc.tensor.matmul(G, lhsT=spge_xn, rhs=pr["ohC_S"], start=False, stop=False)
        nc.tensor.matmul(G, lhsT=splt_xn, rhs=pr["ohC_E"], start=False, stop=False)
        nc.tensor.matmul(G, lhsT=band_xs, rhs=pr["ohC_S"], start=False, stop=True)

        pa = work.tile([P, C], F32, tag="pa")
        rowsum = work.tile([P, 1], F32, tag="rowsum")
        nc.vector.scalar_tensor_tensor(out=pa, in0=G, scalar=1.0, in1=probs,
                                       op0=OP.add, op1=OP.mult, accum_out=rowsum)
        tot = work.tile([P, 1], F32, tag="tot")
        nc.gpsimd.partition_all_reduce(tot, rowsum, channels=P, reduce_op=bass_isa.ReduceOp.add)
        tinv = work.tile([P, 1], F32, tag="tinv")
        nc.vector.reciprocal(tinv, tot)
        res = work.tile([P, C], F32, tag="res")
        nc.scalar.activation(out=res, in_=pa, func=AF.Copy, scale=tinv)
        nc.scalar.dma_start(out=out[b], in_=res)

    # software pipeline: span_prep(0) is emitted first (it runs while the first
    # doc chunk streams in); apply(b-1) is emitted after chunk 0 of matvec(b)
    # and prep(b+1) after chunk 1, so every engine always has queued work while
    # a previous example's cross-engine reduction chain completes.
    preps = {}
    if stop_after != "scores":
        preps[0] = span_prep(0)
    prev = None
    score_tiles = {}
    for b in range(BL):
        scores = work.tile([P, C], F32, tag="scores")
        score_tiles[b] = scores
        for g in range(NCH):
            matvec_chunk(b, g, scores, dtile=pre_dtile if (b == 0 and g == 0) else None,
                         split_last=(b == BL - 1 and g == NCH - 1))
            if stop_after == "scores":
                continue
            if g == min(0, NCH - 1) and prev is not None:
                phases23(prev[0], prev[1], preps.pop(prev[0]))
                prev = None
            if g == min(1, NCH - 1) and b + 1 < BL:
                preps[b + 1] = span_prep(b + 1)
        if stop_after == "scores":
            nc.sync.dma_start(out=out[b], in_=scores)
            continue
        if prev is not None:  # NCH == 1 edge
            phases23(prev[0], prev[1], preps.pop(prev[0]))
        prev = (b, scores)
    if prev is not None and stop_after != "scores":
        phases23(prev[0], prev[1], preps.pop(prev[0]))


def build_nc(z: Sizes, stop_after=None):
    nc = bacc.Bacc("TRN2", target_bir_lowering=False)
    doc = nc.dram_tensor("doc", [z.BL, z.T, z.D], F32, kind="ExternalInput")
    query = nc.dram_tensor("query", [z.BL, z.TQ, z.D], F32, kind="ExternalInput")
    mask = nc.dram_tensor("mask", [z.BL, z.T], F32, kind="ExternalInput")
    spans = nc.dram_tensor("spans", [z.BL, z.S, 2], I32, kind="ExternalInput")
    sent = nc.dram_tensor("sent", [z.BL, z.S], F32, kind="ExternalInput")
    out = nc.dram_tensor("out", [z.BL, P, z.C], F32, kind="ExternalOutput")
    with tile.TileContext(nc) as tc:
        with ExitStack() as ctx:
            build_kernel(ctx, tc, z, doc, query, mask, spans, sent, out, stop_after=stop_after)
    nc.compile()
    return nc


_NC_CACHE = {}


def _get_nc(z: Sizes):
    if z not in _NC_CACHE:
        _NC_CACHE[z] = build_nc(z)
    return _NC_CACHE[z]


def make_in_maps(z: Sizes, n_cores, document_encoded, query_encoded, document_mask, spans, sentence_scores):
    in_maps = []
    for i in range(n_cores):
        lo, hi = i * z.BL, (i + 1) * z.BL
        in_maps.append({
            "doc": np.ascontiguousarray(document_encoded[lo:hi], dtype=np.float32),
            "query": np.ascontiguousarray(query_encoded[lo:hi], dtype=np.float32),
            "mask": np.ascontiguousarray(document_mask[lo:hi], dtype=np.float32),
            "spans": np.ascontiguousarray(spans[lo:hi], dtype=np.int32),
            "sent": np.ascontiguousarray(sentence_scores[lo:hi], dtype=np.float32),
        })
    return in_maps


def kernel(document_encoded, query_encoded, document_mask, spans, sentence_scores, _trace=False):
    document_encoded = np.asarray(document_encoded)
    B, T, D = document_encoded.shape
    TQ = np.asarray(query_encoded).shape[1]
    S = np.asarray(spans).shape[1]
    n_cores = 8
    z = Sizes(BL=B // n_cores, T=T, TQ=TQ, D=D, S=S)
    nc = _get_nc(z)
    in_maps = make_in_maps(z, n_cores, document_encoded, query_encoded, document_mask,
                           spans, sentence_scores)
    r = run_bass_kernel_spmd(nc, in_maps, core_ids=list(range(n_cores)), trace=_trace)
    outs = [r.results[i]["out"].transpose(0, 2, 1).reshape(z.BL, T) for i in range(n_cores)]
    full = np.concatenate(outs, axis=0).astype(np.float32)
    if _trace:
        kernel.last_results = r
    return full
